# revision 13
# baseline (speedup 1.0000x reference)
"""DML (Chamfer-style) L1 loss kernel for Trainium2, 8 NeuronCores.

Math: for each batch b:
  pred2gt_min[j] = min_i ||pred[b,i] - gt[b,j]||_1       (queries = gt)
  gt2pred_min[j] = min_i ||gt_valid[b,i] - pred[b,j]||_1 (queries = pred)
  out = (mean(pred2gt_min) + mean(gt2pred_min)) / 2

Device mapping: data-parallel over B across 8 cores; 4 batches x 2 sides
= 8 "batch-sides" per core.  Rotate coords 45 deg on host (u = x+y,
v = x-y) so L1 dist = max(|du|, |dv|).  Queries and candidates are sorted
by u on host; query chunk c (128 queries) scans only the candidate-rank
window [128c-W, 128c+127+W] (nwin = 128+2W wide).  A host-side
certificate (u-gap to the nearest excluded candidate) verifies each
device minimum and recomputes the provably-uncertain ones exactly, so
the result is exact for any nwin.

Default mode 'seg' (nwin=128, cpb=4 chunks per block):
  - TensorE: one K=12 bf16 ones-matmul per chunk writes du|dv
      [128, 2*nwin] into a half-bank PSUM slot (u/v split 3-way into
      bf16 parts; all products carry a 1.0 factor so PSUM f32 gets du,dv
      exact to ~5e-8).  cpb chunks share a [128, cpb, 256] PSUM tile
      from a 4-deep pool (full pipelining against the consumers).
  - ScalarE: one batched Abs per block moves dv -> SBUF (the DVE may
      read at most one PSUM stream).
  - VectorE: ONE custom-DVE instruction per block: a hand-edited
      segmented min-scan (SEGMIN_ABSMAX1_ANT) computes
      min over window of max(|du|, |dva|) for all cpb chunks, resetting
      the running min at each SUB_DIM_DONE page boundary, and writes the
      page minima directly into the [128, NCH] mins tile through a
      stride-0-inner broadcast out AP (last write per page wins).
  - Host: certificate check + exact fallback, means in float64.
"""
import os
import numpy as np

import concourse.bacc as bacc
import concourse.mybir as mybir
import concourse.tile as tile
from concourse.bass_utils import run_bass_kernel_spmd

F32 = mybir.dt.float32
BF16 = mybir.dt.bfloat16
B, PNUM, D = 32, 2048, 2
NCORES = 8
BPC = B // NCORES          # batches per core
NSIDES = 2 * BPC           # batch-sides per core
P = 128                    # SBUF partitions
NCH = PNUM // P            # query chunks per batch-side
KS = 6                     # matmul contraction: 3 bf16 splits x 2 operands
BIG = 3.0e38

_CACHED = {}


def _register_fused_op():
    """Per-NEFF custom DVE op:
        out = max(max(in0, -in0), in1);  accum_out = min(s0, min(out))
    i.e. a fused |du| + max + min-reduce (the TensorTensorReduce the
    stock ucode lacks, with the abs folded in).
    """
    import concourse.dve_ops as dve_ops
    name = "MIN_OF_ABSMAX_ANT"
    if "fused_op" in _CACHED:
        return _CACHED["fused_op"]
    for o in dve_ops.OPS:
        if o.name == name:
            _CACHED["fused_op"] = o
            return o
    from concourse.dve_spec import Spec, Src0, Src1, C0, Zero, maxx, minn, lower
    from concourse.dve_uop import DveOpSpec

    spec = Spec(body=maxx(maxx(Src0, Zero - Src0), Src1), accum=minn, accum_init=C0)
    row = max(dve_ops._SUB_OPCODE_FOR_NAME.values()) + 1
    assert row < 0x20, "no free custom-DVE opcode rows"
    dve_ops._SUB_OPCODE_FOR_NAME[name] = row
    shas = {}
    for ver in ("v3", "v4"):
        tmp = DveOpSpec(name=name, opcode=row, uops=lower(spec, ver=ver), rd1_en=True)
        shas[ver] = tmp.sha(ver)
    op = dve_ops.DveOp(name, spec, subdim=False, uops_sha=shas)
    dve_ops.OPS.append(op)
    dve_ops.CUSTOM_DVE_SPECS[name] = spec
    _CACHED["fused_op"] = op
    return op


def _register_segmin_op():
    """Hand-edited custom DVE op SEGMIN_ABSMAX1_ANT:
        streams in0 (du, PSUM) and in1 (|dv|, SBUF — pre-abs'ed by ACT;
        the DVE reads at most one PSUM stream) with 3D [P, S, N] APs;
        value = min over j<=k of max(|in0[p,s,j]|, in1[p,s,j]), with the
        running min RESET at each page (subdim) boundary.
    Lower a plain global-scan spec, then add a SUB_DIM_DONE step state
    that reseeds the scan feedback flop from CONST_0 (s0=BIG) while
    consuming the first element of the new page — a segmented min-reduce,
    one instruction per S chunks.  `out` is a stride-0-inner broadcast AP
    over the [P, S] mins tile: the hardware writes the running min every
    element to the same per-page address, so the LAST write (= the page
    minimum) survives — no separate extraction pass."""
    import copy
    import concourse.dve_ops as dve_ops
    from concourse.dve_spec import Spec, Src0, Src1, C0, Zero, maxx, lower, scan, AluOp
    from concourse.dve_uop import DveOpSpec, AluInp, Trigger

    name = "SEGMIN_ABSMAX1_ANT"
    if "segop" in _CACHED:
        return _CACHED["segop"]
    for o in dve_ops.OPS:
        if o.name == name:
            _CACHED["segop"] = o
            return o

    e = maxx(maxx(Src0, Zero - Src0), Src1)
    spec = Spec(body=scan(AluOp.MIN, e, init=C0))
    row = max(dve_ops._SUB_OPCODE_FOR_NAME.values()) + 1
    assert row < 0x20, "no free custom-DVE opcode rows"
    dve_ops._SUB_OPCODE_FOR_NAME[name] = row

    shas = {}
    for ver in ("v3", "v4"):
        uops = lower(spec, ver=ver)
        assert len(uops) == 2, f"expected [seed, steady], got {len(uops)}"
        seed, steady = uops
        scan_blk = None
        for bi, blk in enumerate(steady.datapath_config):
            if blk.op == AluOp.MIN and (
                blk.alu_src0 == AluInp.CURR_ALU_OUT
                or blk.alu_src1 == AluInp.CURR_ALU_OUT
            ):
                scan_blk = bi
                break
        assert scan_blk is not None, "scan combine block not found"
        const_inp = seed.datapath_config[scan_blk].alu_src0
        assert AluInp.PREV_DELAY_0 <= const_inp <= AluInp.PREV_DELAY_0 + 5
        steady.trigger = (Trigger.SRC_TENSOR_DONE, Trigger.SUB_DIM_DONE, Trigger.NONE)
        steady.next_uop = (0, 2, 0)
        step = copy.deepcopy(steady)
        step.trigger = (Trigger.SRC_TENSOR_DONE, Trigger.SUB_DIM_DONE, Trigger.COUNT)
        step.next_uop = (0, 2, 1)
        step.repeat_count = 1
        blk = step.datapath_config[scan_blk]
        if blk.alu_src0 == AluInp.CURR_ALU_OUT:
            blk.alu_src0 = const_inp
        else:
            blk.alu_src1 = const_inp
        edited = DveOpSpec(name=name, opcode=row, uops=[seed, steady, step],
                           rd1_en=True)
        edited.validate(ver)
        shas[ver] = edited.sha(ver)
        dve_ops._COMPILE_CACHE[(name, ver)] = edited

    op = dve_ops.DveOp(name, spec, subdim=True, uops_sha=shas)
    dve_ops.OPS.append(op)
    dve_ops.CUSTOM_DVE_SPECS[name] = spec
    _CACHED["segop"] = op
    return op


def _host_prep_segc(pred, gt, gt_valid, cs, wid):
    """Small-chunk prep: queries and candidates u-sorted; chunk = cs
    consecutive query ranks; window = [cs*c - wid, cs*c + cs - 1 + wid]
    (nwin = cs + 2*wid).  Each 128-partition tile packs cpt = 128//cs
    chunks via a block-diagonal matmul: shared rows 0:3 carry -u_q splits
    (paired with ones in every du column) and rows 3:6 carry -v_q splits
    (ones in every dv column); per chunk-in-tile g, rows 6+6g:9+6g carry
    ones on g's queries (paired with g's candidate-u splits in the du
    columns) and rows 9+6g:12+6g likewise for v.  K = 6 + 6*cpt.
    Returns (in_maps, certs)."""
    import ml_dtypes
    bf = ml_dtypes.bfloat16
    pred = np.asarray(pred, dtype=np.float32)
    gt = np.asarray(gt, dtype=np.float32)
    gt_valid = np.asarray(gt_valid, dtype=np.float32)
    nwin = cs + 2 * wid
    cpt = P // cs
    K = 6 + 6 * cpt
    nch = PNUM // cs
    rhs_cols = NCH * 2 * nwin          # NCH = 16 tiles per side
    base = np.arange(nch)[:, None] * cs - wid + np.arange(nwin)[None, :]
    widx = np.clip(base, 0, PNUM - 1)  # [nch, nwin]
    # chunk-in-tile membership for lhsT ones rows: [cpt, PNUM]
    member = (np.arange(PNUM) % P) // cs  # chunk-in-tile of each rank
    in_maps = []
    certs = []
    for core in range(NCORES):
        pemat = np.zeros((NSIDES, K, PNUM + rhs_cols), bf)
        core_cert = []
        for i in range(BPC):
            b = core * BPC + i
            for side in range(2):
                s = i * 2 + side
                if side == 0:
                    cand, query = pred[b], gt[b]
                else:
                    cand, query = gt_valid[b], pred[b]
                u_c = cand[:, 0] + cand[:, 1]
                v_c = cand[:, 0] - cand[:, 1]
                u_q = query[:, 0] + query[:, 1]
                v_q = query[:, 0] - query[:, 1]
                qord = np.argsort(u_q, kind="stable")
                cord = np.argsort(u_c, kind="stable")
                u_qs, v_qs = u_q[qord], v_q[qord]
                u_cs, v_cs = u_c[cord], v_c[cord]
                a = _split3_bf16(-u_qs)
                e = _split3_bf16(-v_qs)
                for r in range(3):
                    pemat[s, r, :PNUM] = a[r]
                    pemat[s, 3 + r, :PNUM] = e[r]
                for g in range(cpt):
                    on = (member == g).astype(bf)
                    pemat[s, 6 + 6 * g:9 + 6 * g, :PNUM] = on
                    pemat[s, 9 + 6 * g:12 + 6 * g, :PNUM] = on
                ucw = _split3_bf16(u_cs[widx])   # each [nch, nwin]
                vcw = _split3_bf16(v_cs[widx])
                # rhs [K, NCH tiles, 2*nwin]
                rhs = np.zeros((K, NCH, 2 * nwin), bf)
                rhs[0:3, :, :nwin] = 1.0
                rhs[3:6, :, nwin:] = 1.0
                for g in range(cpt):
                    chunks = np.arange(NCH) * cpt + g   # global chunk ids
                    for r in range(3):
                        rhs[6 + 6 * g + r, :, :nwin] = ucw[r][chunks]
                        rhs[9 + 6 * g + r, :, nwin:] = vcw[r][chunks]
                pemat[s, :, PNUM:] = rhs.reshape(K, rhs_cols)
                core_cert.append((u_qs, u_cs, v_qs, v_cs))
        in_maps.append({"pemat": pemat})
        certs.append(core_cert)
    return in_maps, certs


def _build_seg(nwin: int, cpb: int = 4, repeat: int = 1, krows: int = 12):
    """Segmented-scan kernel: per cpb-chunk block, cpb K=12 matmuls fill one
    PSUM tile (one 512-f32 bank slot per chunk: du at [0,nwin), dv at
    [nwin,2nwin)); ONE batched ACT Abs moves dv→SBUF (the DVE reads at most
    one PSUM stream); ONE segmented-min DVE instruction reduces all cpb
    chunks, writing the page minima straight into the mins tile via a
    stride-0 out AP.  Per-instruction fixed costs amortize over cpb chunks."""
    assert NCH % cpb == 0 and 2 * nwin <= 512
    slot = 256 if 2 * nwin <= 256 else 512
    nbufs = max(2, 4096 // (cpb * slot))  # PSUM pipeline depth (8 banks total)
    nc = bacc.Bacc("TRN2", target_bir_lowering=False)
    rhs_cols = NCH * 2 * nwin
    pemat = nc.dram_tensor(
        "pemat", [NSIDES, krows, PNUM + rhs_cols], BF16, kind="ExternalInput"
    )
    outm = nc.dram_tensor("mins", [NSIDES, P, NCH], F32, kind="ExternalOutput")
    segop = _register_segmin_op()

    with tile.TileContext(nc) as tc:
        with (
            tc.tile_pool(name="inp", bufs=2) as inp,
            tc.tile_pool(name="work", bufs=4) as work,
            tc.tile_pool(name="outp", bufs=2) as outp,
            tc.tile_pool(name="ps", bufs=nbufs, space="PSUM") as ps,
        ):
            for rep in range(repeat):
                for s in range(NSIDES):
                    pm = inp.tile([krows, PNUM + rhs_cols], BF16, tag="pm")
                    if s == 0 and rep == 0:
                        # fill-latency path: SP HWDGE (fast start) in two
                        # pieces so block-0 matmuls unblock after piece one
                        cut = PNUM + rhs_cols // 4
                        nc.sync.dma_start(out=pm[:, :cut], in_=pemat[s][:, :cut])
                        nc.sync.dma_start(out=pm[:, cut:], in_=pemat[s][:, cut:])
                    else:
                        nc.gpsimd.dma_start(out=pm[:], in_=pemat[s])
                    mq = outp.tile([P, NCH], F32, tag="mq")
                    # smaller lead blocks on the first side shorten the
                    # DMA->MM->ACT->DVE fill chain
                    if s == 0 and rep == 0 and cpb == 4:
                        blocks = [1, 1, 2] + [cpb] * ((NCH - 4) // cpb)
                    else:
                        blocks = [cpb] * (NCH // cpb)
                    c0 = 0
                    for nb in blocks:
                        duv = ps.tile([P, cpb, slot], F32, tag="duv")
                        for j in range(nb):
                            c = c0 + j
                            nc.tensor.matmul(
                                duv[:, j, 0:2 * nwin],
                                pm[:, c * P:(c + 1) * P],
                                pm[:, PNUM + c * 2 * nwin:PNUM + (c + 1) * 2 * nwin],
                                start=True,
                                stop=True,
                            )
                        dva = work.tile([P, cpb, nwin], F32, tag="dva")
                        nc.scalar.activation(
                            out=dva[:, 0:nb, :],
                            in_=duv[:, 0:nb, nwin:2 * nwin],
                            func=mybir.ActivationFunctionType.Abs,
                            bias=0.0,
                            scale=1.0,
                        )
                        mq_bc = mq[:, c0:c0 + nb].rearrange(
                            "p (s n) -> p s n", n=1
                        ).broadcast_to([P, nb, nwin])
                        nc.vector._custom_dve(
                            segop, out=mq_bc, in0=duv[:, 0:nb, 0:nwin],
                            in1=dva[:, 0:nb, :], s0=BIG,
                        )
                        c0 += nb
                    nc.sync.dma_start(out=outm[s], in_=mq[:])
    nc.compile()
    return nc


def _build_seg2(nwin: int, cpb: int = 8, repeat: int = 1):
    """Like _build_seg but du and dv go to SEPARATE PSUM pools via two K=6
    matmuls per chunk (same pemat: du uses rows 0:6, dv rows 6:12).  This
    splits the PSUM anti-dependency cycle — du-matmuls wait only on the DVE,
    dv-matmuls only on ACT — so blocks can be twice as large (cpb=8 at
    nwin<=128: [P,cpb,slot] = 2 banks per pool buffer, bufs=2 each)."""
    assert NCH % cpb == 0 and nwin <= 256
    slot = 128 if nwin <= 128 else 256
    nbufs = max(2, 2048 // (cpb * slot))  # per-pool PSUM budget: 4 banks
    nc = bacc.Bacc("TRN2", target_bir_lowering=False)
    rhs_cols = NCH * 2 * nwin
    hcols = PNUM + rhs_cols  # u-half / v-half column count
    # [6, 2*hcols]: u rows in cols [0:hcols], v rows in cols [hcols:2*hcols]
    # (both matmul operand sets start at partition 0; one DMA per side)
    pemat = nc.dram_tensor(
        "pemat", [NSIDES, 6, 2 * hcols], BF16, kind="ExternalInput"
    )
    outm = nc.dram_tensor("mins", [NSIDES, P, NCH], F32, kind="ExternalOutput")
    segop = _register_segmin_op()

    with tile.TileContext(nc) as tc:
        with (
            tc.tile_pool(name="inp", bufs=5) as inp,
            tc.tile_pool(name="work", bufs=4) as work,
            tc.tile_pool(name="outp", bufs=6) as outp,
            tc.tile_pool(name="psu", bufs=nbufs, space="PSUM") as psu,
            tc.tile_pool(name="psv", bufs=nbufs, space="PSUM") as psv,
        ):
            for rep in range(repeat):
                for s in range(NSIDES):
                    pm = inp.tile([6, 2 * hcols], BF16, tag="pm")
                    if s == 0 and rep == 0:
                        # fast fill: SP HWDGE; piece 1 covers block-0 operands
                        cut = PNUM + rhs_cols // 4
                        nc.sync.dma_start(out=pm[:, :cut], in_=pemat[s][:, :cut])
                        nc.sync.dma_start(
                            out=pm[:, hcols:hcols + cut],
                            in_=pemat[s][:, hcols:hcols + cut],
                        )
                        nc.sync.dma_start(out=pm[:, cut:hcols],
                                          in_=pemat[s][:, cut:hcols])
                        nc.sync.dma_start(out=pm[:, hcols + cut:],
                                          in_=pemat[s][:, hcols + cut:])
                    else:
                        nc.gpsimd.dma_start(out=pm[:], in_=pemat[s])
                    mq = outp.tile([P, NCH], F32, tag="mq")
                    for b in range(NCH // cpb):
                        dut = psu.tile([P, cpb, slot], F32, tag="dut")
                        dvt = psv.tile([P, cpb, slot], F32, tag="dvt")
                        for j in range(cpb):
                            c = b * cpb + j
                            col = PNUM + c * 2 * nwin
                            nc.tensor.matmul(
                                dut[:, j, 0:nwin],
                                pm[:, c * P:(c + 1) * P],
                                pm[:, col:col + nwin],
                                start=True, stop=True,
                            )
                            nc.tensor.matmul(
                                dvt[:, j, 0:nwin],
                                pm[:, hcols + c * P:hcols + (c + 1) * P],
                                pm[:, hcols + col + nwin:hcols + col + 2 * nwin],
                                start=True, stop=True,
                            )
                        dva = work.tile([P, cpb, nwin], F32, tag="dva")
                        nc.scalar.activation(
                            out=dva[:],
                            in_=dvt[:, :, 0:nwin],
                            func=mybir.ActivationFunctionType.Abs,
                            bias=0.0,
                            scale=1.0,
                        )
                        mq_bc = mq[:, b * cpb:(b + 1) * cpb].rearrange(
                            "p (s n) -> p s n", n=1
                        ).broadcast_to([P, cpb, nwin])
                        nc.vector._custom_dve(
                            segop, out=mq_bc, in0=dut[:, :, 0:nwin],
                            in1=dva[:], s0=BIG,
                        )
                    nc.sync.dma_start(out=outm[s], in_=mq[:])
    nc.compile()
    return nc


def _build_seg3(nwin: int, cpb: int = 8, repeat: int = 1):
    """seg2 with asymmetric PSUM pipelining: du tiles [P,cpb,slot] from a
    3-deep pool (so next-side du matmuls never wait on the previous side's
    DVE), dv tiles in half-blocks [P,cpb//2,slot] from a 2-deep pool (ACT
    paces itself, off the DVE critical chain).  ACT runs twice per DVE
    block, assembling dva in SBUF; the DVE reads du (PSUM) + dva (SBUF)."""
    assert NCH % cpb == 0 and cpb % 2 == 0 and nwin <= 256
    slot = 128 if nwin <= 128 else 256
    hb = cpb // 2
    assert cpb * slot <= 1365 and hb * slot <= 1024  # 3*du + 2*dv <= 8 banks
    nc = bacc.Bacc("TRN2", target_bir_lowering=False)
    rhs_cols = NCH * 2 * nwin
    hcols = PNUM + rhs_cols
    pemat = nc.dram_tensor(
        "pemat", [NSIDES, 6, 2 * hcols], BF16, kind="ExternalInput"
    )
    outm = nc.dram_tensor("mins", [NSIDES, P, NCH], F32, kind="ExternalOutput")
    segop = _register_segmin_op()

    with tile.TileContext(nc) as tc:
        with (
            tc.tile_pool(name="inp", bufs=4) as inp,
            tc.tile_pool(name="work", bufs=4) as work,
            tc.tile_pool(name="outp", bufs=4) as outp,
            tc.tile_pool(name="psu", bufs=3, space="PSUM") as psu,
            tc.tile_pool(name="psv", bufs=2, space="PSUM") as psv,
        ):
            for rep in range(repeat):
                for s in range(NSIDES):
                    pm = inp.tile([6, 2 * hcols], BF16, tag="pm")
                    if s == 0 and rep == 0:
                        cut = PNUM + rhs_cols // 4
                        nc.sync.dma_start(out=pm[:, :cut], in_=pemat[s][:, :cut])
                        nc.sync.dma_start(
                            out=pm[:, hcols:hcols + cut],
                            in_=pemat[s][:, hcols:hcols + cut],
                        )
                        nc.sync.dma_start(out=pm[:, cut:hcols],
                                          in_=pemat[s][:, cut:hcols])
                        nc.sync.dma_start(out=pm[:, hcols + cut:],
                                          in_=pemat[s][:, hcols + cut:])
                    else:
                        nc.gpsimd.dma_start(out=pm[:], in_=pemat[s])
                    mq = outp.tile([P, NCH], F32, tag="mq")
                    for b in range(NCH // cpb):
                        dut = psu.tile([P, cpb, slot], F32, tag="dut")
                        for j in range(cpb):
                            c = b * cpb + j
                            col = PNUM + c * 2 * nwin
                            nc.tensor.matmul(
                                dut[:, j, 0:nwin],
                                pm[:, c * P:(c + 1) * P],
                                pm[:, col:col + nwin],
                                start=True, stop=True,
                            )
                        dva = work.tile([P, cpb, nwin], F32, tag="dva")
                        for h in range(2):
                            dvt = psv.tile([P, hb, slot], F32, tag="dvt")
                            for j2 in range(hb):
                                c = b * cpb + h * hb + j2
                                col = PNUM + c * 2 * nwin
                                nc.tensor.matmul(
                                    dvt[:, j2, 0:nwin],
                                    pm[:, hcols + c * P:hcols + (c + 1) * P],
                                    pm[:, hcols + col + nwin:
                                        hcols + col + 2 * nwin],
                                    start=True, stop=True,
                                )
                            nc.scalar.activation(
                                out=dva[:, h * hb:(h + 1) * hb, :],
                                in_=dvt[:, :, 0:nwin],
                                func=mybir.ActivationFunctionType.Abs,
                                bias=0.0,
                                scale=1.0,
                            )
                        mq_bc = mq[:, b * cpb:(b + 1) * cpb].rearrange(
                            "p (s n) -> p s n", n=1
                        ).broadcast_to([P, cpb, nwin])
                        nc.vector._custom_dve(
                            segop, out=mq_bc, in0=dut[:, :, 0:nwin],
                            in1=dva[:], s0=BIG,
                        )
                    nc.sync.dma_start(out=outm[s], in_=mq[:])
    nc.compile()
    return nc


def _build_segr(nwin: int, cpb: int = 8, repeat: int = 1, cs: int = 16):
    """Resident-layout segmented kernel (cs-rank chunks, u/v split matmuls).

    Per side one [krows, scols] bf16 pemat: krows = 3 + 3*cpt rows
    (query 3-splits on rows 0:3, block-diagonal chunk-membership ones on
    rows 3:, identical for u and v halves), scols = 2*PNUM lhsT columns
    (u then v) followed by per-block interleaved rhs windows (block b:
    cpb u-windows then cpb dv-windows).  Each tile c needs two K=krows
    matmuls (du, dv) of N=nwin into separate PSUM pools ([P, cpb, 128]
    f32 = 2 banks, 2 bufs each).  One batched ACT Abs moves dv to SBUF;
    one segmented-min DVE per block writes page minima into a single
    resident [P, NSIDES*NCH] mins tile; ONE output DMA at the end."""
    cpt = P // cs
    krows = 3 + 3 * cpt
    assert NCH % cpb == 0 and nwin <= 128
    nblk = NCH // cpb
    W = cpb * nwin                  # rhs columns per block per u/v
    scols = 2 * PNUM + 2 * W * nblk
    # column layout per side: [v-rhs_b0 | v-lhsT | u-rhs_b0 | u-lhsT |
    #   (v-rhs_b, u-rhs_b) for b>=1] — puts the side-0 critical prefix
    # (v data for block 0) in one small leading piece.
    col_vl = W
    col_ul = 2 * W + PNUM

    def col_vr(b):
        return 0 if b == 0 else 2 * W + 2 * PNUM + (b - 1) * 2 * W

    def col_ur(b):
        return W + PNUM if b == 0 else col_vr(b) + W

    nc = bacc.Bacc("TRN2", target_bir_lowering=False)
    pemat = nc.dram_tensor(
        "pemat", [NSIDES, krows, scols], BF16, kind="ExternalInput"
    )
    outm = nc.dram_tensor("mins", [P, NSIDES * NCH], F32, kind="ExternalOutput")
    segop = _register_segmin_op()

    with tile.TileContext(nc) as tc:
        with (
            tc.tile_pool(name="inp", bufs=5) as inp,
            tc.tile_pool(name="work", bufs=4) as work,
            tc.tile_pool(name="outp", bufs=1) as outp,
            tc.tile_pool(name="psu", bufs=2, space="PSUM") as psu,
            tc.tile_pool(name="psv", bufs=2, space="PSUM") as psv,
        ):
            mq = outp.tile([P, NSIDES * NCH], F32, tag="mq")
            # PE p-state warm-up: a dependency-free dummy matmul at t~0
            # starts the tensor engine's ramp clock so the first real
            # matmuls (~3us later, after the fill DMA) run at full speed.
            wrm = outp.tile([1, 1], BF16, tag="wrm")
            nc.vector.memset(wrm[:], 1.0)
            dut0 = psu.tile([P, cpb, 128], F32, tag="dut")
            nc.tensor.matmul(dut0[0:1, 0, 120:121], wrm[:], wrm[:],
                             start=True, stop=True)
            pm_tiles = {}
            dv_tiles = {}

            def emit_dma(rep, s):
                pm = inp.tile([krows, scols], BF16, tag="pm")
                pm_tiles[s] = pm
                if s == 0 and rep == 0:
                    # 3-piece fast-start fill: v block-0 data first
                    c1, c2 = W + PNUM, 2 * W + 2 * PNUM
                    nc.sync.dma_start(out=pm[:, :c1], in_=pemat[s][:, :c1])
                    nc.sync.dma_start(out=pm[:, c1:c2], in_=pemat[s][:, c1:c2])
                    nc.sync.dma_start(out=pm[:, c2:], in_=pemat[s][:, c2:])
                else:
                    # sync (SP HWDGE) keeps transfer order aligned with
                    # compute order on the FIFO DMA_ENGINES resource;
                    # v-piece first so the dv->ACT->DVE chain unblocks
                    # before the full side has landed
                    cv = W + PNUM
                    nc.sync.dma_start(out=pm[:, :cv], in_=pemat[s][:, :cv])
                    nc.sync.dma_start(out=pm[:, cv:], in_=pemat[s][:, cv:])

            def emit_dv(rep, k):
                s, b = divmod(k, nblk)
                if b == 0:
                    emit_dma(rep, s)
                pm = pm_tiles[s]
                dvt = psv.tile([P, cpb, 128], F32, tag="dvt")
                dv_tiles[k] = dvt
                for j in range(cpb):
                    c = b * cpb + j
                    nc.tensor.matmul(
                        dvt[:, j, 0:nwin],
                        pm[:, col_vl + c * P:col_vl + (c + 1) * P],
                        pm[:, col_vr(b) + j * nwin:col_vr(b) + (j + 1) * nwin],
                        start=True, stop=True,
                    )

            def emit_du_act_dve(rep, k):
                s, b = divmod(k, nblk)
                pm = pm_tiles[s]
                dvt = dv_tiles.pop(k)
                if s == 0 and rep == 0 and b == 0:
                    dut = dut0
                else:
                    dut = psu.tile([P, cpb, 128], F32, tag="dut")
                for j in range(cpb):
                    c = b * cpb + j
                    nc.tensor.matmul(
                        dut[:, j, 0:nwin],
                        pm[:, col_ul + c * P:col_ul + (c + 1) * P],
                        pm[:, col_ur(b) + j * nwin:col_ur(b) + (j + 1) * nwin],
                        start=True, stop=True,
                    )
                dva = work.tile([P, cpb, nwin], F32, tag="dva")
                nc.scalar.activation(
                    out=dva[:],
                    in_=dvt[:, :, 0:nwin],
                    func=mybir.ActivationFunctionType.Abs,
                    bias=0.0,
                    scale=1.0,
                )
                mq_bc = mq[:, s * NCH + b * cpb:s * NCH + (b + 1) * cpb
                           ].rearrange("p (s n) -> p s n", n=1
                                       ).broadcast_to([P, cpb, nwin])
                nc.vector._custom_dve(
                    segop, out=mq_bc, in0=dut[:, :, 0:nwin],
                    in1=dva[:], s0=BIG,
                )

            # software pipeline: dv-block k+2 is issued before du-block k so
            # neither du's psu-release wait nor ACT's psv-release wait ever
            # stalls a later dv in the in-order PE queue (every dependency
            # cycle drops below the DVE's per-block service time)
            LOOK = 2
            nb_tot = NSIDES * nblk
            for rep in range(repeat):
                for k in range(min(LOOK, nb_tot)):
                    emit_dv(rep, k)
                for k in range(nb_tot):
                    if k + LOOK < nb_tot:
                        emit_dv(rep, k + LOOK)
                    emit_du_act_dve(rep, k)
            nc.sync.dma_start(out=outm[:], in_=mq[:])
    nc.compile()
    return nc


def _host_prep_segr(pred, gt, gt_valid, cs, wid, cpb):
    """Prep for _build_segr. Returns (in_maps, certs)."""
    import ml_dtypes
    bf = ml_dtypes.bfloat16
    pred = np.asarray(pred, dtype=np.float32)
    gt = np.asarray(gt, dtype=np.float32)
    gt_valid = np.asarray(gt_valid, dtype=np.float32)
    nwin = cs + 2 * wid
    cpt = P // cs
    krows = 3 + 3 * cpt
    nch = PNUM // cs                   # rank chunks (128 at cs=16)
    nblk = NCH // cpb
    W = cpb * nwin
    scols = 2 * PNUM + 2 * W * nblk
    col_vl = W
    col_ul = 2 * W + PNUM
    vr_off = [0] + [2 * W + 2 * PNUM + (b - 1) * 2 * W for b in range(1, nblk)]
    ur_off = [W + PNUM] + [vr_off[b] + W for b in range(1, nblk)]
    base = np.arange(nch)[:, None] * cs - wid + np.arange(nwin)[None, :]
    widx = np.clip(base, 0, PNUM - 1)  # [nch, nwin]
    # constant block-diagonal ones rows [3*cpt, PNUM]
    member = (np.arange(PNUM) % P) // cs       # chunk-in-tile of each rank
    onesblk = np.zeros((3 * cpt, PNUM), bf)
    for g in range(cpt):
        onesblk[3 * g:3 * g + 3] = (member == g).astype(bf)
    in_maps = []
    certs = []
    for core in range(NCORES):
        pemat = np.zeros((NSIDES, krows, scols), bf)
        core_cert = []
        for i in range(BPC):
            b = core * BPC + i
            for side in range(2):
                s = i * 2 + side
                if side == 0:
                    cand, query = pred[b], gt[b]
                else:
                    cand, query = gt_valid[b], pred[b]
                u_c = cand[:, 0] + cand[:, 1]
                v_c = cand[:, 0] - cand[:, 1]
                u_q = query[:, 0] + query[:, 1]
                v_q = query[:, 0] - query[:, 1]
                qord = np.argsort(u_q, kind="stable")
                cord = np.argsort(u_c, kind="stable")
                u_qs, v_qs = u_q[qord], v_q[qord]
                u_cs, v_cs = u_c[cord], v_c[cord]
                a = _split3_bf16(-u_qs)
                e = _split3_bf16(-v_qs)
                for r in range(3):
                    pemat[s, r, col_ul:col_ul + PNUM] = a[r]
                    pemat[s, r, col_vl:col_vl + PNUM] = e[r]
                pemat[s, 3:krows, col_vl:col_vl + PNUM] = onesblk
                pemat[s, 3:krows, col_ul:col_ul + PNUM] = onesblk
                ucw = _split3_bf16(u_cs[widx])   # each [nch, nwin]
                vcw = _split3_bf16(v_cs[widx])
                # rhs windows [krows, W] per (block, u/v)
                for blk in range(nblk):
                    for uv, (off, cw) in enumerate(
                        [(vr_off[blk], vcw), (ur_off[blk], ucw)]
                    ):
                        r0 = np.zeros((krows, cpb, nwin), bf)
                        r0[0:3] = 1.0
                        for g in range(cpt):
                            chunks = (blk * cpb + np.arange(cpb)) * cpt + g
                            for r in range(3):
                                r0[3 + 3 * g + r] = cw[r][chunks]
                        pemat[s, :, off:off + W] = r0.reshape(krows, W)
                core_cert.append((u_qs, u_cs, v_qs, v_cs))
        in_maps.append({"pemat": pemat})
        certs.append(core_cert)
    return in_maps, certs


def _build_win(nwin: int, repeat: int = 1):
    """Windowed kernel: queries and candidates sorted by u on host; query
    chunk c scans only the candidate-rank window [128c-W, 128c+127+W]
    (clipped; fixed width nwin). Exactness is certified on the host.

    Inputs per core:
      pemat [NSIDES, 6, PNUM + NCH*nwin] bf16 - lhsT query 3-splits then
            per-chunk candidate-u window 3-splits
      vwin  [NSIDES, 3, NCH*nwin] bf16 - per-chunk candidate-v 3-splits
            (broadcast to 128 partitions via a K=3 ones-matmul)
      vqneg [NSIDES, P, NCH] f32 - ACT bias (-v_q, sorted order)
    Output: mins [NSIDES, P, NCH] f32 (sorted query order).
    """
    nc = bacc.Bacc("TRN2", target_bir_lowering=False)
    rhs_cols = NCH * nwin
    pemat = nc.dram_tensor(
        "pemat", [NSIDES, KS, PNUM + rhs_cols], BF16, kind="ExternalInput"
    )
    vwin = nc.dram_tensor("vwin", [NSIDES, 3, rhs_cols], BF16, kind="ExternalInput")
    vqneg = nc.dram_tensor("vqneg", [NSIDES, P, NCH], F32, kind="ExternalInput")
    outm = nc.dram_tensor("mins", [NSIDES, P, NCH], F32, kind="ExternalOutput")
    fop = _register_fused_op()

    with tile.TileContext(nc) as tc:
        with (
            tc.tile_pool(name="ones", bufs=1) as onep,
            tc.tile_pool(name="inp", bufs=2) as inp,
            tc.tile_pool(name="work", bufs=6) as work,
            tc.tile_pool(name="outp", bufs=2) as outp,
            tc.tile_pool(name="ps", bufs=4, space="PSUM") as ps,
            tc.tile_pool(name="ps2", bufs=4, space="PSUM") as ps2,
        ):
            ones3 = onep.tile([3, P], BF16)
            nc.vector.memset(ones3[:], 1.0)
            for rep in range(repeat):
                for s in range(NSIDES):
                    pm = inp.tile([KS, PNUM + rhs_cols], BF16, tag="pm")
                    nc.gpsimd.dma_start(out=pm[:], in_=pemat[s])
                    vw = inp.tile([3, rhs_cols], BF16, tag="vw")
                    nc.gpsimd.dma_start(out=vw[:], in_=vwin[s])
                    vq = inp.tile([P, NCH], F32, tag="vq")
                    nc.gpsimd.dma_start(out=vq[:], in_=vqneg[s])
                    mq = outp.tile([P, NCH], F32, tag="mq")
                    for c in range(NCH):
                        du = ps.tile([P, nwin], F32, tag="du")
                        nc.tensor.matmul(
                            du[:],
                            pm[:, c * P:(c + 1) * P],
                            pm[:, PNUM + c * nwin:PNUM + (c + 1) * nwin],
                            start=True,
                            stop=True,
                        )
                        vbc = ps2.tile([P, nwin], F32, tag="vbc")
                        nc.tensor.matmul(
                            vbc[:],
                            ones3[:],
                            vw[:, c * nwin:(c + 1) * nwin],
                            start=True,
                            stop=True,
                        )
                        dva = work.tile([P, nwin], F32, tag="dva")
                        nc.scalar.activation(
                            out=dva[:],
                            in_=vbc[:],
                            func=mybir.ActivationFunctionType.Abs,
                            bias=vq[:, c:c + 1],
                            scale=1.0,
                        )
                        dmx = work.tile([P, nwin], F32, tag="dmx")
                        nc.vector._custom_dve(
                            fop, out=dmx[:], in0=du[:], in1=dva[:],
                            s0=BIG, accum_out=mq[:, c:c + 1],
                        )
                    nc.sync.dma_start(out=outm[s], in_=mq[:])
    nc.compile()
    return nc


def _build_win2(nwin: int, repeat: int = 1):
    """Like _build_win but one K=12 matmul per chunk computes both du and
    dv (signed) into one PSUM tile [P, 2*nwin]:
      lhsT rows: [-u_q splits(3), ones(3), -v_q splits(3), ones(3)]
      rhs du-cols: [1,1,1, b0,b1,b2, 0...]; dv-cols: [0..., 1,1,1, d0,d1,d2]
    ACT: |dv| = Abs(dv_psum) -> SBUF.  DVE: fused min-of-absmax.
    Inputs per core: pemat [NSIDES, 12, PNUM + NCH*2*nwin] bf16 only.
    """
    K12 = 12
    nc = bacc.Bacc("TRN2", target_bir_lowering=False)
    rhs_cols = NCH * 2 * nwin
    pemat = nc.dram_tensor(
        "pemat", [NSIDES, K12, PNUM + rhs_cols], BF16, kind="ExternalInput"
    )
    outm = nc.dram_tensor("mins", [NSIDES, P, NCH], F32, kind="ExternalOutput")
    fop = _register_fused_op()

    with tile.TileContext(nc) as tc:
        with (
            tc.tile_pool(name="inp", bufs=2) as inp,
            tc.tile_pool(name="work", bufs=6) as work,
            tc.tile_pool(name="outp", bufs=2) as outp,
            tc.tile_pool(name="ps", bufs=4, space="PSUM") as ps,
        ):
            for rep in range(repeat):
                for s in range(NSIDES):
                    pm = inp.tile([K12, PNUM + rhs_cols], BF16, tag="pm")
                    nc.gpsimd.dma_start(out=pm[:], in_=pemat[s])
                    mq = outp.tile([P, NCH], F32, tag="mq")
                    for c in range(NCH):
                        duv = ps.tile([P, 2 * nwin], F32, tag="duv")
                        nc.tensor.matmul(
                            duv[:],
                            pm[:, c * P:(c + 1) * P],
                            pm[:, PNUM + c * 2 * nwin:PNUM + (c + 1) * 2 * nwin],
                            start=True,
                            stop=True,
                        )
                        dva = work.tile([P, nwin], F32, tag="dva")
                        nc.scalar.activation(
                            out=dva[:],
                            in_=duv[:, nwin:2 * nwin],
                            func=mybir.ActivationFunctionType.Abs,
                            bias=0.0,
                            scale=1.0,
                        )
                        dmx = work.tile([P, nwin], F32, tag="dmx")
                        nc.vector._custom_dve(
                            fop, out=dmx[:], in0=duv[:, 0:nwin], in1=dva[:],
                            s0=BIG, accum_out=mq[:, c:c + 1],
                        )
                    nc.sync.dma_start(out=outm[s], in_=mq[:])
    nc.compile()
    return nc


def _build(repeat: int = 1):
    nc = bacc.Bacc("TRN2", target_bir_lowering=False)
    pemat = nc.dram_tensor("pemat", [NSIDES, KS, 2 * PNUM], BF16, kind="ExternalInput")
    vcand = nc.dram_tensor("vcand", [NSIDES, PNUM], F32, kind="ExternalInput")
    vqneg = nc.dram_tensor("vqneg", [NSIDES, P, NCH], F32, kind="ExternalInput")
    outm = nc.dram_tensor("mins", [NSIDES, P, NCH], F32, kind="ExternalOutput")
    fop = _register_fused_op()

    with tile.TileContext(nc) as tc:
        with (
            tc.tile_pool(name="inp", bufs=2) as inp,
            tc.tile_pool(name="work", bufs=3) as work,
            tc.tile_pool(name="outp", bufs=2) as outp,
            tc.tile_pool(name="ps", bufs=2, space="PSUM") as ps,
        ):
          for rep in range(repeat):
            for s in range(NSIDES):
                pm = inp.tile([KS, 2 * PNUM], BF16, tag="pm")
                nc.gpsimd.dma_start(out=pm[:], in_=pemat[s])
                vr = inp.tile([P, PNUM], F32, tag="vr")
                nc.gpsimd.dma_start(
                    out=vr[:], in_=vcand[s][None, :].broadcast_to([P, PNUM])
                )
                vq = inp.tile([P, NCH], F32, tag="vq")
                nc.gpsimd.dma_start(out=vq[:], in_=vqneg[s])
                mq = outp.tile([P, NCH], F32, tag="mq")
                for c in range(NCH):
                    du = ps.tile([P, PNUM], F32, tag="du")
                    for n in range(4):
                        nc.tensor.matmul(
                            du[:, n * 512:(n + 1) * 512],
                            pm[:, c * P:(c + 1) * P],
                            pm[:, PNUM + n * 512:PNUM + (n + 1) * 512],
                            start=True,
                            stop=True,
                        )
                    dva = work.tile([P, PNUM], F32, tag="dva")
                    nc.scalar.activation(
                        out=dva[:],
                        in_=vr[:],
                        func=mybir.ActivationFunctionType.Abs,
                        bias=vq[:, c:c + 1],
                        scale=1.0,
                    )
                    dmx = work.tile([P, PNUM], F32, tag="dmx")
                    nc.vector._custom_dve(
                        fop, out=dmx[:], in0=du[:], in1=dva[:],
                        s0=BIG, accum_out=mq[:, c:c + 1],
                    )
                nc.sync.dma_start(out=outm[s], in_=mq[:])
    nc.compile()
    return nc


def _mode():
    """(mode, nwin): mode 'segr' (default), 'segc', 'seg', 'seg2', 'seg3',
    'win2', 'win', or 'full'."""
    m = os.environ.get("DML_MODE", "segr")
    nwin = int(os.environ.get("DML_NWIN", "80"))
    return m, nwin


def _segc_cs():
    return int(os.environ.get("DML_CS", "16"))


def _get_nc(repeat: int = 1):
    m, nwin = _mode()
    key = ("nc", m, nwin, repeat)
    if key not in _CACHED:
        if m == "segr":
            cpb = int(os.environ.get("DML_CPB", "8"))
            cs = _segc_cs()
            _CACHED[key] = _build_segr(nwin, cpb, repeat, cs)
        elif m == "seg":
            cpb = int(os.environ.get("DML_CPB", "4"))
            _CACHED[key] = _build_seg(nwin, cpb, repeat)
        elif m == "segc":
            cpb = int(os.environ.get("DML_CPB", "4"))
            cs = _segc_cs()
            _CACHED[key] = _build_seg(nwin, cpb, repeat,
                                      krows=6 + 6 * (P // cs))
        elif m == "seg2":
            cpb = int(os.environ.get("DML_CPB", "8"))
            _CACHED[key] = _build_seg2(nwin, cpb, repeat)
        elif m == "seg3":
            cpb = int(os.environ.get("DML_CPB", "8" if nwin <= 128 else "4"))
            _CACHED[key] = _build_seg3(nwin, cpb, repeat)
        else:
            builder = {"win": _build_win, "win2": _build_win2}.get(m)
            _CACHED[key] = builder(nwin, repeat) if builder else _build(repeat)
    return _CACHED[key]


def _split3_bf16(x):
    """3-way bf16 split: x ~ s0+s1+s2 with ~2^-27 relative residual."""
    import ml_dtypes
    bf = ml_dtypes.bfloat16
    x = x.astype(np.float32)
    s0 = x.astype(bf)
    r1 = x - s0.astype(np.float32)
    s1 = r1.astype(bf)
    r2 = r1 - s1.astype(np.float32)
    s2 = r2.astype(bf)
    return s0, s1, s2


def _host_prep(pred, gt, gt_valid):
    import ml_dtypes
    bf = ml_dtypes.bfloat16
    pred = np.asarray(pred, dtype=np.float32)
    gt = np.asarray(gt, dtype=np.float32)
    gt_valid = np.asarray(gt_valid, dtype=np.float32)
    ones = np.ones(PNUM, bf)
    in_maps = []
    for core in range(NCORES):
        pemat = np.zeros((NSIDES, KS, 2 * PNUM), bf)
        vcand = np.empty((NSIDES, PNUM), np.float32)
        vqneg = np.empty((NSIDES, P, NCH), np.float32)
        for i in range(BPC):
            b = core * BPC + i
            for side in range(2):
                s = i * 2 + side
                if side == 0:   # pred2gt: candidates pred, queries gt
                    cand, query = pred[b], gt[b]
                else:           # gt2pred: candidates gt_valid, queries pred
                    cand, query = gt_valid[b], pred[b]
                u_c = cand[:, 0] + cand[:, 1]
                v_c = cand[:, 0] - cand[:, 1]
                u_q = query[:, 0] + query[:, 1]
                v_q = query[:, 0] - query[:, 1]
                a0, a1, a2 = _split3_bf16(-u_q)
                b0, b1, b2 = _split3_bf16(u_c)
                # lhsT half (queries): rows [-a0,-a1,-a2, 1,1,1]
                pemat[s, 0, :PNUM] = a0
                pemat[s, 1, :PNUM] = a1
                pemat[s, 2, :PNUM] = a2
                pemat[s, 3, :PNUM] = ones
                pemat[s, 4, :PNUM] = ones
                pemat[s, 5, :PNUM] = ones
                # rhs half (candidates): rows [1,1,1, b0,b1,b2]
                pemat[s, 0, PNUM:] = ones
                pemat[s, 1, PNUM:] = ones
                pemat[s, 2, PNUM:] = ones
                pemat[s, 3, PNUM:] = b0
                pemat[s, 4, PNUM:] = b1
                pemat[s, 5, PNUM:] = b2
                vcand[s] = v_c
                vqneg[s] = (-v_q).reshape(NCH, P).T
        in_maps.append({"pemat": pemat, "vcand": vcand, "vqneg": vqneg})
    return in_maps


def _host_prep_win(pred, gt, gt_valid, nwin):
    """Sorted-window prep. Returns (in_maps, certs) where certs[core][s] =
    (u_q_sorted, u_c_sorted, v_q_sorted, cand_sorted_uv, query_sorted_uv)
    for the exactness certificate + fallback."""
    import ml_dtypes
    bf = ml_dtypes.bfloat16
    pred = np.asarray(pred, dtype=np.float32)
    gt = np.asarray(gt, dtype=np.float32)
    gt_valid = np.asarray(gt_valid, dtype=np.float32)
    W = (nwin - P) // 2
    rhs_cols = NCH * nwin
    onesP = np.ones(PNUM, bf)
    in_maps = []
    certs = []
    # per-chunk candidate rank windows (shared across sides): ranks clipped
    base = np.arange(NCH)[:, None] * P - W + np.arange(nwin)[None, :]
    widx = np.clip(base, 0, PNUM - 1)          # [NCH, nwin]
    for core in range(NCORES):
        pemat = np.zeros((NSIDES, KS, PNUM + rhs_cols), bf)
        vwin = np.zeros((NSIDES, 3, rhs_cols), bf)
        vqneg = np.empty((NSIDES, P, NCH), np.float32)
        core_cert = []
        for i in range(BPC):
            b = core * BPC + i
            for side in range(2):
                s = i * 2 + side
                if side == 0:   # pred2gt: candidates pred, queries gt
                    cand, query = pred[b], gt[b]
                else:           # gt2pred: candidates gt_valid, queries pred
                    cand, query = gt_valid[b], pred[b]
                u_c = cand[:, 0] + cand[:, 1]
                v_c = cand[:, 0] - cand[:, 1]
                u_q = query[:, 0] + query[:, 1]
                v_q = query[:, 0] - query[:, 1]
                qord = np.argsort(u_q, kind="stable")
                cord = np.argsort(u_c, kind="stable")
                u_qs, v_qs = u_q[qord], v_q[qord]
                u_cs, v_cs = u_c[cord], v_c[cord]
                a0, a1, a2 = _split3_bf16(-u_qs)
                pemat[s, 0, :PNUM] = a0
                pemat[s, 1, :PNUM] = a1
                pemat[s, 2, :PNUM] = a2
                pemat[s, 3:6, :PNUM] = onesP
                ucw = u_cs[widx].reshape(-1)     # [NCH*nwin]
                vcw = v_cs[widx].reshape(-1)
                c0, c1, c2 = _split3_bf16(ucw)
                pemat[s, 0:3, PNUM:] = 1.0
                pemat[s, 3, PNUM:] = c0
                pemat[s, 4, PNUM:] = c1
                pemat[s, 5, PNUM:] = c2
                d0, d1, d2 = _split3_bf16(vcw)
                vwin[s, 0] = d0
                vwin[s, 1] = d1
                vwin[s, 2] = d2
                vqneg[s] = (-v_qs).reshape(NCH, P).T
                core_cert.append((u_qs, u_cs, v_qs, v_cs))
        in_maps.append({"pemat": pemat, "vwin": vwin, "vqneg": vqneg})
        certs.append(core_cert)
    return in_maps, certs


def _host_prep_win2(pred, gt, gt_valid, nwin):
    """Prep for _build_win2: one bf16 pemat per core with K=12 rows.
    Returns (in_maps, certs); certs identical to _host_prep_win."""
    import ml_dtypes
    bf = ml_dtypes.bfloat16
    pred = np.asarray(pred, dtype=np.float32)
    gt = np.asarray(gt, dtype=np.float32)
    gt_valid = np.asarray(gt_valid, dtype=np.float32)
    W = (nwin - P) // 2
    rhs_cols = NCH * 2 * nwin
    in_maps = []
    certs = []
    base = np.arange(NCH)[:, None] * P - W + np.arange(nwin)[None, :]
    widx = np.clip(base, 0, PNUM - 1)          # [NCH, nwin]
    for core in range(NCORES):
        pemat = np.zeros((NSIDES, 12, PNUM + rhs_cols), bf)
        core_cert = []
        for i in range(BPC):
            b = core * BPC + i
            for side in range(2):
                s = i * 2 + side
                if side == 0:
                    cand, query = pred[b], gt[b]
                else:
                    cand, query = gt_valid[b], pred[b]
                u_c = cand[:, 0] + cand[:, 1]
                v_c = cand[:, 0] - cand[:, 1]
                u_q = query[:, 0] + query[:, 1]
                v_q = query[:, 0] - query[:, 1]
                qord = np.argsort(u_q, kind="stable")
                cord = np.argsort(u_c, kind="stable")
                u_qs, v_qs = u_q[qord], v_q[qord]
                u_cs, v_cs = u_c[cord], v_c[cord]
                # lhsT [12, PNUM]: -u_q splits, ones, -v_q splits, ones
                a = _split3_bf16(-u_qs)
                e = _split3_bf16(-v_qs)
                for r in range(3):
                    pemat[s, r, :PNUM] = a[r]
                    pemat[s, 6 + r, :PNUM] = e[r]
                pemat[s, 3:6, :PNUM] = 1.0
                pemat[s, 9:12, :PNUM] = 1.0
                # rhs: per chunk, du cols then dv cols
                bspl = _split3_bf16(u_cs[widx])     # each [NCH, nwin]
                dspl = _split3_bf16(v_cs[widx])
                rhs = np.zeros((12, NCH, 2 * nwin), bf)
                rhs[0:3, :, :nwin] = 1.0
                for r in range(3):
                    rhs[3 + r, :, :nwin] = bspl[r]
                rhs[6:9, :, nwin:] = 1.0
                for r in range(3):
                    rhs[9 + r, :, nwin:] = dspl[r]
                pemat[s, :, PNUM:] = rhs.reshape(12, rhs_cols)
                core_cert.append((u_qs, u_cs, v_qs, v_cs))
        in_maps.append({"pemat": pemat})
        certs.append(core_cert)
    return in_maps, certs


def _certify_and_fix(mins_dev, certs, nwin, cs=P):
    """mins_dev: [cores, NSIDES, P, NCH] device window-minima in sorted-query
    order (query rank r = tile*P + p). Verify each against the u-gap to the
    nearest excluded candidate; recompute failures exactly. Returns
    (mins_fixed flat [cores, NSIDES, PNUM], n_fallback)."""
    W = (nwin - cs) // 2
    out = np.empty((len(certs), NSIDES, PNUM), np.float64)
    n_fb = 0
    ranks = np.arange(PNUM)
    chunk = ranks // cs
    lo_eff = np.maximum(chunk * cs - W, 0)                 # [PNUM]
    hi_eff = np.minimum(chunk * cs + (cs - 1) + W, PNUM - 1)
    for ci, core_cert in enumerate(certs):
        for s, (u_qs, u_cs, v_qs, v_cs) in enumerate(core_cert):
            m = mins_dev[ci, s].T.reshape(-1).astype(np.float64)  # rank order
            gap_l = np.where(
                lo_eff > 0, u_qs - u_cs[np.maximum(lo_eff - 1, 0)], np.inf
            )
            gap_r = np.where(
                hi_eff < PNUM - 1, u_cs[np.minimum(hi_eff + 1, PNUM - 1)] - u_qs,
                np.inf,
            )
            bad = m > np.minimum(gap_l, gap_r)
            if bad.any():
                n_fb += int(bad.sum())
                uq, vq = u_qs[bad], v_qs[bad]
                du = np.abs(u_cs[None, :] - uq[:, None])
                dv = np.abs(v_cs[None, :] - vq[:, None])
                m[bad] = np.maximum(du, dv).min(axis=1)
            out[ci, s] = m
    return out, n_fb


def kernel(pred, gt, gt_valid, loss_type, _want_results=False):
    assert int(loss_type) == 1, f"only L1 supported, got {loss_type}"
    m, nwin = _mode()
    nc = _get_nc()
    if m == "segr":
        cs = _segc_cs()
        cpb = int(os.environ.get("DML_CPB", "8"))
        assert (nwin - cs) % 2 == 0
        in_maps, certs = _host_prep_segr(
            pred, gt, gt_valid, cs, (nwin - cs) // 2, cpb
        )
    elif m == "win":
        in_maps, certs = _host_prep_win(pred, gt, gt_valid, nwin)
    elif m == "segc":
        cs = _segc_cs()
        assert (nwin - cs) % 2 == 0
        in_maps, certs = _host_prep_segc(pred, gt, gt_valid, cs, (nwin - cs) // 2)
    elif m in ("win2", "seg", "seg2", "seg3"):
        in_maps, certs = _host_prep_win2(pred, gt, gt_valid, nwin)
        if m in ("seg2", "seg3"):
            # repack [NSIDES, 12, hcols] -> [NSIDES, 6, 2*hcols]: v rows
            # (6:12) concatenated after u rows (0:6) along columns
            for im in in_maps:
                pe = im["pemat"]
                im["pemat"] = np.concatenate([pe[:, 0:6], pe[:, 6:12]], axis=2)
    else:
        in_maps = _host_prep(pred, gt, gt_valid)
        certs = None
    res = run_bass_kernel_spmd(
        nc, in_maps, core_ids=list(range(NCORES)),
        trace=os.environ.get("DML_TRACE", "0") == "1",
    )
    mins = np.stack([res.results[c]["mins"] for c in range(NCORES)])
    if m == "segr":
        # [cores, P, NSIDES*NCH] -> [cores, NSIDES, P, NCH]
        mins = mins.reshape(len(mins), P, NSIDES, NCH).transpose(0, 2, 1, 3)
    # mins: [cores, NSIDES, P, NCH]; side = s % 2
    if m in ("win", "win2", "seg", "seg2", "seg3", "segc", "segr"):
        cs = _segc_cs() if m in ("segc", "segr") else P
        fixed, n_fb = _certify_and_fix(mins, certs, nwin, cs)
        if os.environ.get("DML_VERBOSE"):
            print(f"[kernel] window fallbacks: {n_fb}")
        m_side = [fixed[:, side::2].mean() for side in range(2)]
    else:
        mins = mins.astype(np.float64)
        m_side = [mins[:, side::2].mean() for side in range(2)]
    out = np.float32((m_side[0] + m_side[1]) / 2.0)
    if _want_results:
        return out, res
    return out

